# revision 15
# baseline (speedup 1.0000x reference)
"""Trainium2 Bass kernel for nn_ARMFeedForward (dense MoE w/ top-2 masked combine).

Sharding: data-parallel over tokens across 8 NeuronCores (1024 tokens/core),
weights replicated, no collectives. Host does layout/dtype prep only
(transpose + bf16 cast); all arithmetic of the module runs on-chip:
  logits = (x @ c_norm^T)/||x|| + x @ w_route^T          (fp32 on PE/DVE/ACT)
  gates  = top2-masked softmax(logits)                    (DVE/ACT)
  out    = sum_e gate_e * (gelu(x@W1_e + b1_e) @ W2_e + b2_e)   (bf16 PE, fp32 PSUM)

Schedule: two 512-token FFN tiles; tile-1 routing is software-pipelined into
tile-0's FFN phases so the PE never idles on the routing chain. DMA traffic:
xt16 + weights (need-ordered: xt16_0, w1[e]s, xt16_1, w2[e]s) stream on the
GpSimd SWDGE queue; x-slices and outputs on the SP HWDGE queue; only the tiny
gate rows use the (slow) Act HWDGE queue.
"""

import sys
from contextlib import ExitStack

import numpy as np

try:
    import concourse  # noqa: F401
except ImportError:
    sys.path.insert(0, "/opt/trn_rl_repo")

import ml_dtypes

import concourse.bass as bass
import concourse.mybir as mybir
import concourse.tile as tile
from concourse import bacc, masks
from concourse.tile_rust import add_dep_helper
from concourse.bass_utils import run_bass_kernel_spmd

F32 = mybir.dt.float32
BF16 = mybir.dt.bfloat16
AF = mybir.ActivationFunctionType
OP = mybir.AluOpType
AX = mybir.AxisListType

N_CORES = 8
B, S, D = 4, 2048, 1024
E, H = 8, 512
N_TOK = B * S              # 8192
T_CORE = N_TOK // N_CORES  # 1024 tokens per core
TT = 512                   # tokens per FFN tile (N=512 matmuls hide LDWEIGHTS)
N_TILES = T_CORE // TT     # 2
NSL = T_CORE // 128        # 8 routing slices of 128 tokens
SPT = TT // 128            # routing slices per FFN tile (4)
DC = D // 128              # 8 contraction chunks over d_model
HC = H // 128              # 4 chunks over expert hidden
NEG_BIG = -1.0e30
GELU_FUNC = AF.Gelu


def build_nc() -> bass.Bass:
    nc = bacc.Bacc("TRN2", target_bir_lowering=False, debug=False)

    # ---- DRAM parameters (per-core shard views, host-prepped layouts) ----
    xt32_d = nc.declare_dram_parameter("xt32", [NSL, 128, DC, 128], F32, isOutput=False)
    xt16_d = nc.declare_dram_parameter("xt16", [N_TILES, 128, DC, TT], BF16, isOutput=False)
    xn_d = nc.declare_dram_parameter("xn", [T_CORE, D], F32, isOutput=False)
    w1_d = nc.declare_dram_parameter("w1b", [E // 2, 128, 2, DC, H], BF16, isOutput=False)
    w2_d = nc.declare_dram_parameter("w2b", [E // 2, 128, 2, HC, D], BF16, isOutput=False)
    b1_d = nc.declare_dram_parameter("b1t", [128, E * HC], F32, isOutput=False)
    b2_d = nc.declare_dram_parameter("b2b", [E, D], BF16, isOutput=False)
    cent_d = nc.declare_dram_parameter("cent", [E, D], F32, isOutput=False)
    wrt_d = nc.declare_dram_parameter("wrt", [E, D], F32, isOutput=False)
    out_d = nc.declare_dram_parameter("out", [T_CORE, D], F32, isOutput=True)

    with tile.TileContext(nc) as tc:
        with ExitStack() as ctx:
            # ---------------- static SBUF tiles ----------------
            statics = ctx.enter_context(tc.tile_pool(name="statics", bufs=1))
            ident = statics.tile([128, 128], F32, tag="ident")
            ones1 = statics.tile([1, 128], BF16, tag="ones1")
            r_sb = statics.tile([128, DC, 2 * E], F32, tag="r_sb")  # [dP, dc, cos|rt]
            b1_sb = statics.tile([128, E * HC], F32, tag="b1_sb")
            b2_sb = statics.tile([E, D], BF16, tag="b2_sb")
            w1p_sb = [
                statics.tile([128, 2, DC, H], BF16, tag=f"w1_{j}", name=f"w1s_{j}")
                for j in range(E // 2)
            ]
            w2p_sb = [
                statics.tile([128, 2, HC, D], BF16, tag=f"w2_{j}", name=f"w2s_{j}")
                for j in range(E // 2)
            ]

            masks.make_identity(nc, ident[:, :])
            nc.vector.memset(ones1[:, :], 1.0)

            # ------------- early DMA triggers -------------
            # SWDGE (GpSimd) queue in need-order: xt16_0, all w1, xt16_1, all w2.
            xt16_p = ctx.enter_context(tc.tile_pool(name="xt16", bufs=2))
            xt16_t = [
                xt16_p.tile([128, DC, TT], BF16, tag="xt16", name=f"xt16_{ti}")
                for ti in range(N_TILES)
            ]
            nc.gpsimd.dma_start(xt16_t[0][:, :, :], xt16_d[0, :, :, :])
            for j in range(E // 2):
                nc.gpsimd.dma_start(w1p_sb[j][:, :, :, :], w1_d[j, :, :, :, :])
            nc.gpsimd.dma_start(xt16_t[1][:, :, :], xt16_d[1, :, :, :])
            for j in range(E // 2):
                nc.gpsimd.dma_start(w2p_sb[j][:, :, :, :], w2_d[j, :, :, :, :])
            # small constants on the SP queue
            nc.sync.dma_start(b1_sb[:, :], b1_d[:, :])
            nc.sync.dma_start(b2_sb[:, :], b2_d[:, :])

            # ------------- centroid normalize + routing matrix R (transient) ----------
            with tc.tile_pool(name="pre", bufs=1) as pre_p, tc.tile_pool(
                name="pre_ps", bufs=2, space="PSUM"
            ) as pre_ps:
                cent_sb = pre_p.tile([E, D], F32, tag="cent_sb")
                wrt_sb = pre_p.tile([E, D], F32, tag="wrt_sb")
                csq_sb = pre_p.tile([E, D], F32, tag="csq_sb")
                cn2 = pre_p.tile([E, 1], F32, tag="cn2")
                crn = pre_p.tile([E, 1], F32, tag="crn")
                nc.sync.dma_start(cent_sb[:, :], cent_d[:, :])
                nc.sync.dma_start(wrt_sb[:, :], wrt_d[:, :])
                # c_norm = centroids / max(||centroids||, eps)
                nc.scalar.activation(
                    csq_sb[:, :], cent_sb[:, :], AF.Square, accum_out=cn2[:, :]
                )
                nc.scalar.activation(cn2[:, :], cn2[:, :], AF.Sqrt)
                nc.vector.tensor_scalar_max(cn2[:, :], cn2[:, :], 1.0e-12)
                nc.vector.reciprocal(crn[:, :], cn2[:, :])
                nc.vector.tensor_scalar(
                    cent_sb[:, :], cent_sb[:, :], crn[:, :], None, op0=OP.mult
                )
                # R[:, dc, 0:8] = c_norm^T chunk, R[:, dc, 8:16] = w_route^T chunk
                for dc in range(DC):
                    tp = pre_ps.tile([128, E], F32, tag="rtp", name=f"rtp_{dc}")
                    nc.tensor.transpose(
                        tp[:, :], cent_sb[:, bass.ts(dc, 128)], ident[0:E, 0:E]
                    )
                    nc.vector.tensor_copy(r_sb[:, dc, 0:E], tp[:, :])
                    tp2 = pre_ps.tile([128, E], F32, tag="rtp", name=f"rtp2_{dc}")
                    nc.tensor.transpose(
                        tp2[:, :], wrt_sb[:, bass.ts(dc, 128)], ident[0:E, 0:E]
                    )
                    nc.vector.tensor_copy(r_sb[:, dc, E : 2 * E], tp2[:, :])

            # ---------------- pools ----------------
            xt32_p = ctx.enter_context(tc.tile_pool(name="xt32", bufs=2))
            xn_p = ctx.enter_context(tc.tile_pool(name="xn", bufs=1))
            sm_p = ctx.enter_context(tc.tile_pool(name="smalls", bufs=2))
            rt_p = ctx.enter_context(tc.tile_pool(name="rt", bufs=SPT + 1))
            gt_p = ctx.enter_context(tc.tile_pool(name="gt", bufs=1))
            gf_p = ctx.enter_context(tc.tile_pool(name="gf", bufs=1))
            hg_p = ctx.enter_context(tc.tile_pool(name="hg", bufs=2))
            hs_p = ctx.enter_context(tc.tile_pool(name="hs", bufs=8))
            osb_p = ctx.enter_context(tc.tile_pool(name="osb", bufs=2))

            lp_ps = ctx.enter_context(tc.tile_pool(name="lp_ps", bufs=2, space="PSUM"))
            gt_ps = ctx.enter_context(tc.tile_pool(name="gt_ps", bufs=1, space="PSUM"))
            h_ps = ctx.enter_context(tc.tile_pool(name="h_ps", bufs=2, space="PSUM"))
            gbc_ps = ctx.enter_context(tc.tile_pool(name="gbc_ps", bufs=1, space="PSUM"))
            oa_ps = ctx.enter_context(tc.tile_pool(name="oa_ps", bufs=2, space="PSUM"))

            def routing_pass1(ti):
                """DMA x slices, norms, fp32 logits -> per-slice lg tiles."""
                lg_t = []
                for sl in range(ti * SPT, (ti + 1) * SPT):
                    ta = sl * 128
                    xt32 = xt32_p.tile(
                        [128, DC, 128], F32, tag="xt32", name=f"xt32_{sl}"
                    )
                    nc.sync.dma_start(xt32[:, :, :], xt32_d[sl, :, :, :])
                    xnat = xn_p.tile([128, D], F32, tag="xnat", name=f"xn_{sl}")
                    nc.sync.dma_start(xnat[:, :], xn_d[ta : ta + 128, :])

                    n2 = sm_p.tile([128, 1], F32, tag="n2", name=f"n2_{sl}")
                    nc.scalar.activation(
                        xnat[:, :], xnat[:, :], AF.Square, accum_out=n2[:, :]
                    )
                    nc.scalar.activation(n2[:, :], n2[:, :], AF.Sqrt)
                    nc.vector.tensor_scalar_max(n2[:, :], n2[:, :], 1.0e-12)
                    rinv = rt_p.tile([128, 1], F32, tag="rinv", name=f"rinv_{sl}")
                    nc.vector.reciprocal(rinv[:, :], n2[:, :])

                    lps = lp_ps.tile([128, 2 * E], F32, tag="lps", name=f"lps_{sl}")
                    for dc in range(DC):
                        nc.tensor.matmul(
                            lps[:, :],
                            xt32[:, dc, :],
                            r_sb[:, dc, :],
                            start=(dc == 0),
                            stop=(dc == DC - 1),
                        )
                    lg = rt_p.tile([128, E], F32, tag="lg", name=f"lg_{sl}")
                    nc.vector.tensor_scalar(
                        lg[:, :], lps[:, 0:E], rinv[:, :], None, op0=OP.mult
                    )
                    nc.vector.tensor_tensor(
                        lg[:, :], lg[:, :], lps[:, E : 2 * E], op=OP.add
                    )
                    lg_t.append(lg)
                return lg_t

            def routing_pass2(ti, lg_t):
                """Top-2 masked softmax + gate transpose -> (gt16, gflat)."""
                gt16 = gt_p.tile([E, TT], BF16, tag="gt16", name=f"gt16_{ti}")
                for k, sl in enumerate(range(ti * SPT, (ti + 1) * SPT)):
                    lg = lg_t[k]
                    m1 = sm_p.tile([128, 1], F32, tag="m1", name=f"m1_{sl}")
                    nc.vector.tensor_reduce(m1[:, :], lg[:, :], axis=AX.X, op=OP.max)
                    nm1 = sm_p.tile([128, 1], F32, tag="nm1", name=f"nm1_{sl}")
                    nc.vector.tensor_scalar(
                        nm1[:, :], m1[:, :], -1.0, None, op0=OP.mult
                    )
                    ee = sm_p.tile([128, E], F32, tag="ee", name=f"ee_{sl}")
                    nc.scalar.activation(
                        ee[:, :], lg[:, :], AF.Exp, bias=nm1[:, :], scale=1.0
                    )

                    nm = sm_p.tile([128, E], F32, tag="nm", name=f"nm_{sl}")
                    nc.vector.tensor_scalar(
                        nm[:, :], lg[:, :], m1[:, :], NEG_BIG,
                        op0=OP.is_equal, op1=OP.mult,
                    )
                    nc.vector.tensor_tensor(nm[:, :], lg[:, :], nm[:, :], op=OP.add)
                    m2 = sm_p.tile([128, 1], F32, tag="m2", name=f"m2_{sl}")
                    nc.vector.tensor_reduce(m2[:, :], nm[:, :], axis=AX.X, op=OP.max)

                    gu = sm_p.tile([128, E], F32, tag="gu", name=f"gu_{sl}")
                    nc.vector.tensor_scalar(
                        gu[:, :], lg[:, :], m2[:, :], None, op0=OP.is_ge
                    )
                    nc.vector.tensor_tensor(gu[:, :], gu[:, :], ee[:, :], op=OP.mult)
                    den = sm_p.tile([128, 1], F32, tag="den", name=f"den_{sl}")
                    nc.vector.tensor_reduce(den[:, :], gu[:, :], axis=AX.X, op=OP.add)
                    rden = sm_p.tile([128, 1], F32, tag="rden", name=f"rden_{sl}")
                    nc.vector.reciprocal(rden[:, :], den[:, :])
                    g = sm_p.tile([128, E], F32, tag="g", name=f"g_{sl}")
                    nc.vector.tensor_scalar(
                        g[:, :], gu[:, :], rden[:, :], None, op0=OP.mult
                    )

                    gtp = gt_ps.tile([E, 128], F32, tag="gtp", name=f"gtp_{sl}")
                    nc.tensor.transpose(gtp[:, :], g[:, :], ident[:, :])
                    last_cp = nc.scalar.copy(gt16[:, bass.ts(k, 128)], gtp[:, :])

                gflat = gf_p.tile([1, E, TT], BF16, tag="gflat", name=f"gflat_{ti}")
                nc.sync.dma_start(gflat[0:1, :, :], gt16[:, :])
                return gt16, gflat, last_cp

            def ffn_phase_a(ti, gflat, act_after=None):
                xt16 = xt16_t[ti]
                hs_all = [
                    hs_p.tile([128, HC, TT], BF16, tag="hs", name=f"hs_{ti}_{e}")
                    for e in range(E)
                ]
                for e in range(E):
                    hps_l = []
                    for hc in range(HC):
                        hps = h_ps.tile(
                            [128, TT], F32, tag="hps", name=f"hps_{ti}_{e}_{hc}"
                        )
                        for dc in range(DC):
                            nc.tensor.matmul(
                                hps[:, :],
                                w1p_sb[e // 2][:, e % 2, dc, bass.ts(hc, 128)],
                                xt16[:, dc, :],
                                start=(dc == 0),
                                stop=(dc == DC - 1),
                            )
                        hps_l.append(hps)
                    gbc = gbc_ps.tile([128, TT], F32, tag="gbc", name=f"gbc_{ti}_{e}")
                    nc.tensor.matmul(
                        gbc[:, :], ones1[0:1, :], gflat[0:1, e, :],
                        start=True, stop=True,
                    )
                    for hc in range(HC):
                        hps = hps_l[hc]
                        hg = hg_p.tile(
                            [128, TT], BF16, tag="hg", name=f"hg_{ti}_{e}_{hc}"
                        )
                        gl = nc.scalar.activation(
                            hg[:, :],
                            hps[:, :],
                            GELU_FUNC,
                            bias=b1_sb[:, e * HC + hc : e * HC + hc + 1],
                        )
                        if act_after is not None:
                            add_dep_helper(
                                gl.ins, act_after.ins, sync=False,
                                reason="gelus after this tile's routing ACT ops",
                            )
                            act_after = None
                        nc.vector.tensor_tensor(
                            hs_all[e][:, hc, :], hg[:, :], gbc[:, :], op=OP.mult
                        )
                return hs_all

            def ffn_phase_b(ti, gt16, hs_all):
                for tsl in range(SPT):
                    ta = ti * TT + tsl * 128
                    oa = [
                        oa_ps.tile(
                            [128, 512], F32, tag="oa", name=f"oa_{ti}_{tsl}_{dh}"
                        )
                        for dh in range(2)
                    ]
                    for e in range(E):
                        for hc in range(HC):
                            for dh in range(2):
                                nc.tensor.matmul(
                                    oa[dh][:, :],
                                    hs_all[e][:, hc, bass.ts(tsl, 128)],
                                    w2p_sb[e // 2][:, e % 2, hc, bass.ts(dh, 512)],
                                    start=(e == 0 and hc == 0),
                                    stop=False,
                                )
                    for dh in range(2):
                        nc.tensor.matmul(
                            oa[dh][:, :],
                            gt16[:, bass.ts(tsl, 128)],
                            b2_sb[:, bass.ts(dh, 512)],
                            start=False,
                            stop=True,
                        )
                        osb = osb_p.tile(
                            [128, 512], F32, tag="osb", name=f"osb_{ti}_{tsl}_{dh}"
                        )
                        nc.scalar.copy(osb[:, :], oa[dh][:, :])
                        nc.sync.dma_start(
                            out_d[ta : ta + 128, bass.ts(dh, 512)], osb[:, :]
                        )

            # ---- software-pipelined schedule: tile-1 routing hides in tile-0 FFN ----
            lg0 = routing_pass1(0)
            gt16_0, gflat_0, cp0 = routing_pass2(0, lg0)
            hs0 = ffn_phase_a(0, gflat_0, act_after=cp0)
            lg1 = routing_pass1(1)
            ffn_phase_b(0, gt16_0, hs0)
            gt16_1, gflat_1, cp1 = routing_pass2(1, lg1)
            hs1 = ffn_phase_a(1, gflat_1, act_after=cp1)
            ffn_phase_b(1, gt16_1, hs1)

    if not nc.is_finalized():
        nc.finalize()
    return nc


def _prep_inputs(x, w1, b1, w2, b2, centroids, w_route):
    """Host-side layout/dtype prep + sharding. Returns per-core in_maps."""
    bf16 = ml_dtypes.bfloat16
    xf = np.ascontiguousarray(x.reshape(N_TOK, D).astype(np.float32))
    # [E//2, 128dp, 2e, DC, H]
    w1b = np.ascontiguousarray(
        w1.astype(np.float32)
        .reshape(E // 2, 2, DC, 128, H)
        .transpose(0, 3, 1, 2, 4)
        .astype(bf16)
    )
    # [E//2, 128hp, 2e, HC, D]
    w2b = np.ascontiguousarray(
        w2.astype(np.float32)
        .reshape(E // 2, 2, HC, 128, D)
        .transpose(0, 3, 1, 2, 4)
        .astype(bf16)
    )
    b1t = np.ascontiguousarray(
        b1.astype(np.float32).reshape(E, HC, 128).transpose(2, 0, 1).reshape(128, E * HC)
    )
    b2b = np.ascontiguousarray(b2.astype(np.float32).astype(bf16))
    cent = np.ascontiguousarray(centroids.astype(np.float32))
    wrt = np.ascontiguousarray(w_route.astype(np.float32))

    in_maps = []
    for c in range(N_CORES):
        xs = xf[c * T_CORE : (c + 1) * T_CORE]            # [1024, 1024]
        xt = np.ascontiguousarray(xs.T)                    # [d, t]
        # [NSL, 128dp, DC, 128t] — contiguous per routing slice
        xt32 = np.ascontiguousarray(
            xt.reshape(DC, 128, NSL, 128).transpose(2, 1, 0, 3)
        )
        # [N_TILES, 128dp, DC, TT] — contiguous per FFN tile
        xt16 = np.ascontiguousarray(
            xt.reshape(DC, 128, N_TILES, TT).transpose(2, 1, 0, 3).astype(bf16)
        )
        in_maps.append(
            {
                "xt32": xt32,
                "xt16": xt16,
                "xn": xs,
                "w1b": w1b,
                "w2b": w2b,
                "b1t": b1t,
                "b2b": b2b,
                "cent": cent,
                "wrt": wrt,
            }
        )
    return in_maps


_CACHE = {}


def kernel(**inputs) -> np.ndarray:
    in_maps = _prep_inputs(
        inputs["x"], inputs["w1"], inputs["b1"], inputs["w2"], inputs["b2"],
        inputs["centroids"], inputs["w_route"],
    )
    if "nc" not in _CACHE:
        _CACHE["nc"] = build_nc()
    res = run_bass_kernel_spmd(_CACHE["nc"], in_maps, core_ids=list(range(N_CORES)))
    out = np.concatenate([res.results[c]["out"] for c in range(N_CORES)], axis=0)
    return np.ascontiguousarray(out.reshape(B, S, D).astype(np.float32))


if __name__ == "__main__":
    rng = np.random.default_rng(0)
    ins = {
        "x": rng.standard_normal((B, S, D), dtype=np.float32),
        "w1": rng.standard_normal((E, D, H), dtype=np.float32) / np.sqrt(D),
        "b1": np.zeros((E, H), np.float32),
        "w2": rng.standard_normal((E, H, D), dtype=np.float32) / np.sqrt(H),
        "b2": np.zeros((E, D), np.float32),
        "centroids": rng.standard_normal((E, D), dtype=np.float32) * 0.02,
        "w_route": rng.standard_normal((E, D), dtype=np.float32),
    }
    out = kernel(**ins)
    print(out.shape, out.dtype)


# revision 18
# speedup vs baseline: 1.0413x; 1.0413x over previous
"""Trainium2 Bass kernel for nn_ARMFeedForward (dense MoE w/ top-2 masked combine).

Sharding: data-parallel over tokens across 8 NeuronCores (1024 tokens/core),
weights replicated, no collectives. Host does layout/dtype prep only
(transpose + bf16 cast); all arithmetic of the module runs on-chip:
  logits = (x @ c_norm^T)/||x|| + x @ w_route^T          (fp32 on PE/DVE/ACT)
  gates  = top2-masked softmax(logits)                    (DVE/ACT)
  out    = sum_e gate_e * (gelu(x@W1_e + b1_e) @ W2_e + b2_e)   (bf16 PE, fp32 PSUM)

Schedule: two 512-token FFN tiles; tile-1 routing is software-pipelined into
tile-0's FFN phases so the PE never idles on the routing chain. DMA traffic:
xt16 + weights (need-ordered: xt16_0, w1[e]s, xt16_1, w2[e]s) stream on the
GpSimd SWDGE queue; x-slices and outputs on the SP HWDGE queue; only the tiny
gate rows use the (slow) Act HWDGE queue.
"""

import sys
from contextlib import ExitStack

import numpy as np

try:
    import concourse  # noqa: F401
except ImportError:
    sys.path.insert(0, "/opt/trn_rl_repo")

import ml_dtypes

import concourse.bass as bass
import concourse.mybir as mybir
import concourse.tile as tile
from concourse import bacc, masks
from concourse.bass_utils import run_bass_kernel_spmd

F32 = mybir.dt.float32
BF16 = mybir.dt.bfloat16
AF = mybir.ActivationFunctionType
OP = mybir.AluOpType
AX = mybir.AxisListType

N_CORES = 8
B, S, D = 4, 2048, 1024
E, H = 8, 512
N_TOK = B * S              # 8192
T_CORE = N_TOK // N_CORES  # 1024 tokens per core
TT = 512                   # tokens per FFN tile (N=512 matmuls hide LDWEIGHTS)
N_TILES = T_CORE // TT     # 2
NSL = T_CORE // 128        # 8 routing slices of 128 tokens
SPT = TT // 128            # routing slices per FFN tile (4)
DC = D // 128              # 8 contraction chunks over d_model
HC = H // 128              # 4 chunks over expert hidden
NEG_BIG = -1.0e30
GELU_FUNC = AF.Gelu


def build_nc() -> bass.Bass:
    nc = bacc.Bacc("TRN2", target_bir_lowering=False, debug=False)

    # ---- DRAM parameters (per-core shard views, host-prepped layouts) ----
    xt32_d = nc.declare_dram_parameter("xt32", [NSL, 128, DC, 128], F32, isOutput=False)
    xt16_d = nc.declare_dram_parameter("xt16", [N_TILES, 128, DC, TT], BF16, isOutput=False)
    xn_d = nc.declare_dram_parameter("xn", [T_CORE, D], F32, isOutput=False)
    w1_d = nc.declare_dram_parameter("w1b", [E // 2, 128, 2, DC, H], BF16, isOutput=False)
    w2_d = nc.declare_dram_parameter("w2b", [E // 2, 128, 2, HC, D], BF16, isOutput=False)
    b1_d = nc.declare_dram_parameter("b1t", [128, E * HC], F32, isOutput=False)
    b2_d = nc.declare_dram_parameter("b2b", [E, D], BF16, isOutput=False)
    cent_d = nc.declare_dram_parameter("cent", [E, D], F32, isOutput=False)
    wrt_d = nc.declare_dram_parameter("wrt", [E, D], F32, isOutput=False)
    out_d = nc.declare_dram_parameter("out", [T_CORE, D], F32, isOutput=True)

    with tile.TileContext(nc) as tc:
        with ExitStack() as ctx:
            # ---------------- static SBUF tiles ----------------
            statics = ctx.enter_context(tc.tile_pool(name="statics", bufs=1))
            ident = statics.tile([128, 128], F32, tag="ident")
            ones1 = statics.tile([1, 128], BF16, tag="ones1")
            r_sb = statics.tile([128, DC, 2 * E], F32, tag="r_sb")  # [dP, dc, cos|rt]
            b1_sb = statics.tile([128, E * HC], F32, tag="b1_sb")
            b2_sb = statics.tile([E, D], BF16, tag="b2_sb")
            w1p_sb = [
                statics.tile([128, 2, DC, H], BF16, tag=f"w1_{j}", name=f"w1s_{j}")
                for j in range(E // 2)
            ]
            w2p_sb = [
                statics.tile([128, 2, HC, D], BF16, tag=f"w2_{j}", name=f"w2s_{j}")
                for j in range(E // 2)
            ]

            masks.make_identity(nc, ident[:, :])
            nc.vector.memset(ones1[:, :], 1.0)

            # ------------- early DMA triggers -------------
            # SWDGE (GpSimd) queue in need-order: xt16_0, all w1, xt16_1, all w2.
            xt16_p = ctx.enter_context(tc.tile_pool(name="xt16", bufs=2))
            xt16_t = [
                xt16_p.tile([128, DC, TT], BF16, tag="xt16", name=f"xt16_{ti}")
                for ti in range(N_TILES)
            ]
            nc.gpsimd.dma_start(xt16_t[0][:, :, :], xt16_d[0, :, :, :])
            for j in range(E // 2):
                nc.gpsimd.dma_start(w1p_sb[j][:, :, :, :], w1_d[j, :, :, :, :])
            nc.gpsimd.dma_start(xt16_t[1][:, :, :], xt16_d[1, :, :, :])
            for j in range(E // 2):
                nc.gpsimd.dma_start(w2p_sb[j][:, :, :, :], w2_d[j, :, :, :, :])
            # small constants on the SP queue
            nc.sync.dma_start(b1_sb[:, :], b1_d[:, :])
            nc.sync.dma_start(b2_sb[:, :], b2_d[:, :])

            # ------------- centroid normalize + routing matrix R (transient) ----------
            with tc.tile_pool(name="pre", bufs=1) as pre_p, tc.tile_pool(
                name="pre_ps", bufs=2, space="PSUM"
            ) as pre_ps:
                cent_sb = pre_p.tile([E, D], F32, tag="cent_sb")
                wrt_sb = pre_p.tile([E, D], F32, tag="wrt_sb")
                csq_sb = pre_p.tile([E, D], F32, tag="csq_sb")
                cn2 = pre_p.tile([E, 1], F32, tag="cn2")
                crn = pre_p.tile([E, 1], F32, tag="crn")
                nc.sync.dma_start(cent_sb[:, :], cent_d[:, :])
                nc.sync.dma_start(wrt_sb[:, :], wrt_d[:, :])
                # c_norm = centroids / max(||centroids||, eps)
                nc.scalar.activation(
                    csq_sb[:, :], cent_sb[:, :], AF.Square, accum_out=cn2[:, :]
                )
                nc.scalar.activation(cn2[:, :], cn2[:, :], AF.Sqrt)
                nc.vector.tensor_scalar_max(cn2[:, :], cn2[:, :], 1.0e-12)
                nc.vector.reciprocal(crn[:, :], cn2[:, :])
                nc.vector.tensor_scalar(
                    cent_sb[:, :], cent_sb[:, :], crn[:, :], None, op0=OP.mult
                )
                # R[:, dc, 0:8] = c_norm^T chunk, R[:, dc, 8:16] = w_route^T chunk
                for dc in range(DC):
                    tp = pre_ps.tile([128, E], F32, tag="rtp", name=f"rtp_{dc}")
                    nc.tensor.transpose(
                        tp[:, :], cent_sb[:, bass.ts(dc, 128)], ident[0:E, 0:E]
                    )
                    nc.vector.tensor_copy(r_sb[:, dc, 0:E], tp[:, :])
                    tp2 = pre_ps.tile([128, E], F32, tag="rtp", name=f"rtp2_{dc}")
                    nc.tensor.transpose(
                        tp2[:, :], wrt_sb[:, bass.ts(dc, 128)], ident[0:E, 0:E]
                    )
                    nc.vector.tensor_copy(r_sb[:, dc, E : 2 * E], tp2[:, :])

            # ---------------- pools ----------------
            xt32_p = ctx.enter_context(tc.tile_pool(name="xt32", bufs=2))
            xn_p = ctx.enter_context(tc.tile_pool(name="xn", bufs=1))
            sm_p = ctx.enter_context(tc.tile_pool(name="smalls", bufs=2))
            rt_p = ctx.enter_context(tc.tile_pool(name="rt", bufs=SPT + 1))
            gt_p = ctx.enter_context(tc.tile_pool(name="gt", bufs=1))
            gf_p = ctx.enter_context(tc.tile_pool(name="gf", bufs=1))
            hs_p = ctx.enter_context(tc.tile_pool(name="hs", bufs=8))
            osb_p = ctx.enter_context(tc.tile_pool(name="osb", bufs=2))

            lp_ps = ctx.enter_context(tc.tile_pool(name="lp_ps", bufs=1, space="PSUM"))
            gt_ps = ctx.enter_context(tc.tile_pool(name="gt_ps", bufs=1, space="PSUM"))
            h_ps = ctx.enter_context(tc.tile_pool(name="h_ps", bufs=2, space="PSUM"))
            gbc_ps = ctx.enter_context(tc.tile_pool(name="gbc_ps", bufs=2, space="PSUM"))
            oa_ps = ctx.enter_context(tc.tile_pool(name="oa_ps", bufs=2, space="PSUM"))

            xt32_tiles = {}

            def routing_norms(ti):
                """DMA x slices + per-token 1/||x|| (no PE work)."""
                rinv_t = {}
                for sl in range(ti * SPT, (ti + 1) * SPT):
                    ta = sl * 128
                    xt32 = xt32_p.tile(
                        [128, DC, 128], F32, tag="xt32", name=f"xt32_{sl}"
                    )
                    nc.sync.dma_start(xt32[:, :, :], xt32_d[sl, :, :, :])
                    xt32_tiles[sl] = xt32
                    xnat = xn_p.tile([128, D], F32, tag="xnat", name=f"xn_{sl}")
                    nc.sync.dma_start(xnat[:, :], xn_d[ta : ta + 128, :])

                    n2 = sm_p.tile([128, 1], F32, tag="n2", name=f"n2_{sl}")
                    nc.scalar.activation(
                        xnat[:, :], xnat[:, :], AF.Square, accum_out=n2[:, :]
                    )
                    nc.scalar.activation(n2[:, :], n2[:, :], AF.Sqrt)
                    nc.vector.tensor_scalar_max(n2[:, :], n2[:, :], 1.0e-12)
                    rinv = rt_p.tile([128, 1], F32, tag="rinv", name=f"rinv_{sl}")
                    nc.vector.reciprocal(rinv[:, :], n2[:, :])
                    rinv_t[sl] = rinv
                return rinv_t

            def routing_logits(sl, rinv):
                """fp32 logits matmuls + combine for one 128-token slice."""
                lps = lp_ps.tile([128, 2 * E], F32, tag="lps", name=f"lps_{sl}")
                for dc in range(DC):
                    nc.tensor.matmul(
                        lps[:, :],
                        xt32_tiles[sl][:, dc, :],
                        r_sb[:, dc, :],
                        start=(dc == 0),
                        stop=(dc == DC - 1),
                    )
                lg = rt_p.tile([128, E], F32, tag="lg", name=f"lg_{sl}")
                nc.vector.tensor_scalar(
                    lg[:, :], lps[:, 0:E], rinv[:, :], None, op0=OP.mult
                )
                nc.vector.tensor_tensor(
                    lg[:, :], lg[:, :], lps[:, E : 2 * E], op=OP.add
                )
                return lg

            def routing_pass2(ti, lg_t):
                """Top-2 masked softmax + gate transpose -> (gt16, gflat)."""
                gt16 = gt_p.tile([E, TT], BF16, tag="gt16", name=f"gt16_{ti}")
                for k, sl in enumerate(range(ti * SPT, (ti + 1) * SPT)):
                    lg = lg_t[sl]
                    m1 = sm_p.tile([128, 1], F32, tag="m1", name=f"m1_{sl}")
                    nc.vector.tensor_reduce(m1[:, :], lg[:, :], axis=AX.X, op=OP.max)
                    nm1 = sm_p.tile([128, 1], F32, tag="nm1", name=f"nm1_{sl}")
                    nc.vector.tensor_scalar(
                        nm1[:, :], m1[:, :], -1.0, None, op0=OP.mult
                    )
                    ee = sm_p.tile([128, E], F32, tag="ee", name=f"ee_{sl}")
                    nc.scalar.activation(
                        ee[:, :], lg[:, :], AF.Exp, bias=nm1[:, :], scale=1.0
                    )

                    nm = sm_p.tile([128, E], F32, tag="nm", name=f"nm_{sl}")
                    nc.vector.tensor_scalar(
                        nm[:, :], lg[:, :], m1[:, :], NEG_BIG,
                        op0=OP.is_equal, op1=OP.mult,
                    )
                    nc.vector.tensor_tensor(nm[:, :], lg[:, :], nm[:, :], op=OP.add)
                    m2 = sm_p.tile([128, 1], F32, tag="m2", name=f"m2_{sl}")
                    nc.vector.tensor_reduce(m2[:, :], nm[:, :], axis=AX.X, op=OP.max)

                    gu = sm_p.tile([128, E], F32, tag="gu", name=f"gu_{sl}")
                    nc.vector.tensor_scalar(
                        gu[:, :], lg[:, :], m2[:, :], None, op0=OP.is_ge
                    )
                    nc.vector.tensor_tensor(gu[:, :], gu[:, :], ee[:, :], op=OP.mult)
                    den = sm_p.tile([128, 1], F32, tag="den", name=f"den_{sl}")
                    nc.vector.tensor_reduce(den[:, :], gu[:, :], axis=AX.X, op=OP.add)
                    rden = sm_p.tile([128, 1], F32, tag="rden", name=f"rden_{sl}")
                    nc.vector.reciprocal(rden[:, :], den[:, :])
                    g = sm_p.tile([128, E], F32, tag="g", name=f"g_{sl}")
                    nc.vector.tensor_scalar(
                        g[:, :], gu[:, :], rden[:, :], None, op0=OP.mult
                    )

                    gtp = gt_ps.tile([E, 128], F32, tag="gtp", name=f"gtp_{sl}")
                    nc.tensor.transpose(gtp[:, :], g[:, :], ident[:, :])
                    nc.scalar.copy(gt16[:, bass.ts(k, 128)], gtp[:, :])

                gflat = gf_p.tile([1, E, TT], BF16, tag="gflat", name=f"gflat_{ti}")
                nc.sync.dma_start(gflat[0:1, :, :], gt16[:, :])
                return gt16, gflat

            def ffn_w1_expert(ti, hs_all, e):
                """w1 matmuls + bias+gelu for one expert, gelu straight into hs."""
                xt16 = xt16_t[ti]
                for hc in range(HC):
                    hps = h_ps.tile(
                        [128, TT], F32, tag="hps", name=f"hps_{ti}_{e}_{hc}"
                    )
                    for dc in range(DC):
                        nc.tensor.matmul(
                            hps[:, :],
                            w1p_sb[e // 2][:, e % 2, dc, bass.ts(hc, 128)],
                            xt16[:, dc, :],
                            start=(dc == 0),
                            stop=(dc == DC - 1),
                        )
                    nc.scalar.activation(
                        hs_all[e][:, hc, :],
                        hps[:, :],
                        GELU_FUNC,
                        bias=b1_sb[:, e * HC + hc : e * HC + hc + 1],
                    )

            def ffn_gate_expert(ti, gflat, hs_all, e):
                """broadcast gate row e and scale hs in place."""
                gbc = gbc_ps.tile([128, TT], F32, tag="gbc", name=f"gbc_{ti}_{e}")
                nc.tensor.matmul(
                    gbc[:, :], ones1[0:1, :], gflat[0:1, e, :],
                    start=True, stop=True,
                )
                for hc in range(HC):
                    nc.vector.tensor_tensor(
                        hs_all[e][:, hc, :], hs_all[e][:, hc, :], gbc[:, :],
                        op=OP.mult,
                    )

            def ffn_phase_b(ti, gt16, hs_all, interleave=None):
                for tsl in range(SPT):
                    ta = ti * TT + tsl * 128
                    oa = [
                        oa_ps.tile(
                            [128, 512], F32, tag="oa", name=f"oa_{ti}_{tsl}_{dh}"
                        )
                        for dh in range(2)
                    ]
                    for e in range(E):
                        for hc in range(HC):
                            for dh in range(2):
                                nc.tensor.matmul(
                                    oa[dh][:, :],
                                    hs_all[e][:, hc, bass.ts(tsl, 128)],
                                    w2p_sb[e // 2][:, e % 2, hc, bass.ts(dh, 512)],
                                    start=(e == 0 and hc == 0),
                                    stop=False,
                                )
                    for dh in range(2):
                        nc.tensor.matmul(
                            oa[dh][:, :],
                            gt16[:, bass.ts(tsl, 128)],
                            b2_sb[:, bass.ts(dh, 512)],
                            start=False,
                            stop=True,
                        )
                        osb = osb_p.tile(
                            [128, 512], F32, tag="osb", name=f"osb_{ti}_{tsl}_{dh}"
                        )
                        nc.scalar.copy(osb[:, :], oa[dh][:, :])
                        nc.sync.dma_start(
                            out_d[ta : ta + 128, bass.ts(dh, 512)], osb[:, :]
                        )
                    if interleave is not None:
                        interleave(tsl)

            # ---- software-pipelined schedule ----
            # tile 0: w1 work starts as soon as weights arrive; routing logits
            # are interleaved between expert blocks (their x arrives on the
            # contended SP queue); gating is deferred until the gate rows are
            # built, overlapping the remaining w1 experts on the DVE.
            rinv0 = routing_norms(0)
            hs0 = [
                hs_p.tile([128, HC, TT], BF16, tag="hs", name=f"hs_0_{e}")
                for e in range(E)
            ]
            lg0 = {}
            ffn_w1_expert(0, hs0, 0)
            ffn_w1_expert(0, hs0, 1)
            for sl in range(0, 2):
                lg0[sl] = routing_logits(sl, rinv0[sl])
            ffn_w1_expert(0, hs0, 2)
            ffn_w1_expert(0, hs0, 3)
            for sl in range(2, 4):
                lg0[sl] = routing_logits(sl, rinv0[sl])
            gt16_0, gflat_0 = routing_pass2(0, lg0)
            for e in range(4, E):
                ffn_w1_expert(0, hs0, e)
            rinv1 = routing_norms(1)
            lg1 = {}
            for e in range(E):
                ffn_gate_expert(0, gflat_0, hs0, e)
                if e % 2 == 1 and 4 + e // 2 < NSL:
                    sl = 4 + e // 2
                    lg1[sl] = routing_logits(sl, rinv1[sl])
            ffn_phase_b(0, gt16_0, hs0)
            gt16_1, gflat_1 = routing_pass2(1, lg1)
            hs1 = [
                hs_p.tile([128, HC, TT], BF16, tag="hs", name=f"hs_1_{e}")
                for e in range(E)
            ]
            for e in range(E):
                ffn_w1_expert(1, hs1, e)
            for e in range(E):
                ffn_gate_expert(1, gflat_1, hs1, e)
            ffn_phase_b(1, gt16_1, hs1)

    if not nc.is_finalized():
        nc.finalize()
    return nc


def _prep_inputs(x, w1, b1, w2, b2, centroids, w_route):
    """Host-side layout/dtype prep + sharding. Returns per-core in_maps."""
    bf16 = ml_dtypes.bfloat16
    xf = np.ascontiguousarray(x.reshape(N_TOK, D).astype(np.float32))
    # [E//2, 128dp, 2e, DC, H]
    w1b = np.ascontiguousarray(
        w1.astype(np.float32)
        .reshape(E // 2, 2, DC, 128, H)
        .transpose(0, 3, 1, 2, 4)
        .astype(bf16)
    )
    # [E//2, 128hp, 2e, HC, D]
    w2b = np.ascontiguousarray(
        w2.astype(np.float32)
        .reshape(E // 2, 2, HC, 128, D)
        .transpose(0, 3, 1, 2, 4)
        .astype(bf16)
    )
    b1t = np.ascontiguousarray(
        b1.astype(np.float32).reshape(E, HC, 128).transpose(2, 0, 1).reshape(128, E * HC)
    )
    b2b = np.ascontiguousarray(b2.astype(np.float32).astype(bf16))
    cent = np.ascontiguousarray(centroids.astype(np.float32))
    wrt = np.ascontiguousarray(w_route.astype(np.float32))

    in_maps = []
    for c in range(N_CORES):
        xs = xf[c * T_CORE : (c + 1) * T_CORE]            # [1024, 1024]
        xt = np.ascontiguousarray(xs.T)                    # [d, t]
        # [NSL, 128dp, DC, 128t] — contiguous per routing slice
        xt32 = np.ascontiguousarray(
            xt.reshape(DC, 128, NSL, 128).transpose(2, 1, 0, 3)
        )
        # [N_TILES, 128dp, DC, TT] — contiguous per FFN tile
        xt16 = np.ascontiguousarray(
            xt.reshape(DC, 128, N_TILES, TT).transpose(2, 1, 0, 3).astype(bf16)
        )
        in_maps.append(
            {
                "xt32": xt32,
                "xt16": xt16,
                "xn": xs,
                "w1b": w1b,
                "w2b": w2b,
                "b1t": b1t,
                "b2b": b2b,
                "cent": cent,
                "wrt": wrt,
            }
        )
    return in_maps


_CACHE = {}


def kernel(**inputs) -> np.ndarray:
    in_maps = _prep_inputs(
        inputs["x"], inputs["w1"], inputs["b1"], inputs["w2"], inputs["b2"],
        inputs["centroids"], inputs["w_route"],
    )
    if "nc" not in _CACHE:
        _CACHE["nc"] = build_nc()
    res = run_bass_kernel_spmd(_CACHE["nc"], in_maps, core_ids=list(range(N_CORES)))
    out = np.concatenate([res.results[c]["out"] for c in range(N_CORES)], axis=0)
    return np.ascontiguousarray(out.reshape(B, S, D).astype(np.float32))


if __name__ == "__main__":
    rng = np.random.default_rng(0)
    ins = {
        "x": rng.standard_normal((B, S, D), dtype=np.float32),
        "w1": rng.standard_normal((E, D, H), dtype=np.float32) / np.sqrt(D),
        "b1": np.zeros((E, H), np.float32),
        "w2": rng.standard_normal((E, H, D), dtype=np.float32) / np.sqrt(H),
        "b2": np.zeros((E, D), np.float32),
        "centroids": rng.standard_normal((E, D), dtype=np.float32) * 0.02,
        "w_route": rng.standard_normal((E, D), dtype=np.float32),
    }
    out = kernel(**ins)
    print(out.shape, out.dtype)
